# revision 6
# baseline (speedup 1.0000x reference)
"""Trainium2 Bass kernel for causal self-attention (B=4, S=2048, C=2048, H=16).

Sharding over 8 NeuronCores: core = 2*batch + head_group
  - data-parallel over the 4 batches (outer axis)
  - tensor-parallel over heads within a batch: 2 groups x 8 heads
Each core computes qkv projection for its head group, block-causal
flash-style attention for its 8 heads, and a partial output projection
(contraction over its 1024 w_proj rows). The host sums the two partial
outputs per batch and adds b_proj ("all-reduce" done during unshard).

Device compute is bf16 with f32 PSUM accumulation. All auxiliary PE
matmuls (row-sum reduce, reciprocal broadcast) use bf16 operands so they
run at 1 cycle/row instead of fp32's 4; the causal diag mask and the v
bias are applied on DVE instead of PE.
"""

from contextlib import ExitStack

import numpy as np
import ml_dtypes

import concourse.bass as bass
import concourse.tile as tile
from concourse import bacc, mybir
from concourse.bass_utils import run_bass_kernel_spmd

BF16 = mybir.dt.bfloat16
F32 = mybir.dt.float32
ExpF = mybir.ActivationFunctionType.Exp
NPBF16 = ml_dtypes.bfloat16

B, S, C, H = 4, 2048, 2048, 16
D = 128
N_CORES = 8
NH = 8              # heads per core
NQ = NH * D         # 1024 q (=k=v) columns per core
SQT = 512           # sq tile width


def _build(compile=True, reps=1):
    CK = C // 128            # contraction chunks
    NST = S // SQT           # s tiles of 512
    NSC = S // 128           # s chunks of 128
    NB_QK = 2 * NQ // 128    # q+k output chunks of 128
    NVT = NQ // 256          # v n-tiles of 256
    ET = C // 512            # proj e tiles
    scale = 1.0 / float(np.sqrt(float(D)))

    nc = bacc.Bacc(
        "TRN2",
        target_bir_lowering=False,
        debug=False,
        enable_asserts=False,
        num_devices=N_CORES,
    )
    xT_d = nc.dram_tensor("xT", [128, NSC * CK * 128], BF16, kind="ExternalInput").ap()
    # weights preswizzled on host: per-partition-contiguous chunk runs
    wqk_d = nc.dram_tensor("wqk", [128, NB_QK * CK * 128], BF16, kind="ExternalInput").ap()
    wv_d = nc.dram_tensor("wv", [128, NVT * CK * 256], BF16, kind="ExternalInput").ap()
    wp_d = nc.dram_tensor("wp", [128, NH * C], BF16, kind="ExternalInput").ap()
    bqkvcol_d = nc.dram_tensor(
        "bqkvcol", [128, NB_QK], BF16, kind="ExternalInput"
    ).ap()
    bvb_d = nc.dram_tensor("bvb", [128, NQ], BF16, kind="ExternalInput").ap()
    utri_d = nc.dram_tensor("utri", [128, 128], BF16, kind="ExternalInput").ap()
    out_d = nc.dram_tensor("out", [S, C], F32, kind="ExternalOutput").ap()

    with tile.TileContext(nc) as tc, ExitStack() as top:
        persist = top.enter_context(tc.tile_pool(name="persist", bufs=1))
        # q_sb/k_sb: [d, h, s]; after attention, yT_h overwrites q_sb[:, h, :]
        q_sb = persist.tile([128, NH, S], BF16, tag="q")
        k_sb = persist.tile([128, NH, S], BF16, tag="k")
        # v_sb: [s%128, s//128, h*128+d], natural v layout per s-chunk
        v_sb = persist.tile([128, NSC, NQ], BF16, tag="v")
        utri_sb = persist.tile([128, 128], BF16, tag="utri")
        # q/k bias as per-partition columns: bias_col[p, nb] = bqkv[nb*128 + p]
        bias_col = persist.tile([128, NB_QK], BF16, tag="bias_col")
        # v bias broadcast across partitions: bvb[p, n] = bqkv_v[n]
        bvb_sb = persist.tile([128, NQ], BF16, tag="bvb")
        ones_col_bf = persist.tile([128, 1], BF16, tag="ones_col_bf")
        ones_row_bf = persist.tile([1, 128], BF16, tag="ones_row_bf")

        nc.gpsimd.dma_start(out=utri_sb, in_=utri_d)
        nc.gpsimd.dma_start(out=bias_col, in_=bqkvcol_d)
        nc.vector.memset(ones_col_bf, 1.0)
        nc.vector.memset(ones_row_bf, 1.0)

        for _rep in range(reps):
            # ---------------- Phase 1: QKV projection ----------------
            # x fully resident; each weight column chunk read exactly once.
            # Section order: v, then k, then q — so attention t=0 unblocks asap.
            with (
                tc.tile_pool(name="ph1x", bufs=1) as ph1x,
                tc.tile_pool(name="ph1wv", bufs=2) as ph1wv,
                tc.tile_pool(name="ph1wqk", bufs=3) as ph1wqk,
                tc.tile_pool(name="ps1", bufs=4, space="PSUM") as ps1,
            ):
                def load_wv(nt, engs):
                    wt = ph1wv.tile([128, CK, 256], BF16, tag="wv")
                    # split along ck so the first matmuls unblock earlier
                    hk = CK // 2
                    src = wv_d[:, nt * CK * 256 : (nt + 1) * CK * 256].rearrange(
                        "p (ck n) -> p ck n", ck=CK
                    )
                    engs[0].dma_start(out=wt[:, :hk, :], in_=src[:, :hk, :])
                    engs[1].dma_start(out=wt[:, hk:, :], in_=src[:, hk:, :])
                    return wt

                # v weights stream on the gpsimd queue (x owns sync/scalar);
                # wv0 is split along ck so the first matmuls unblock sooner
                wt0 = load_wv(0, [nc.gpsimd, nc.gpsimd])
                # v bias broadcast after wv0, before wv1-3: needed by the
                # first DVE copy at ~6us in
                nc.gpsimd.dma_start(out=bvb_sb, in_=bvb_d)
                xfull = ph1x.tile([128, CK, S], BF16, tag="xf")
                dma_engs = [nc.sync, nc.scalar]
                for sc in range(NSC):
                    # slab sc: per-partition contiguous 4KB run from swizzled xT
                    dma_engs[sc % 2].dma_start(
                        out=xfull[:, :, bass.ts(sc, 128)],
                        in_=xT_d[:, sc * CK * 128 : (sc + 1) * CK * 128].rearrange(
                            "p (ck sl) -> p ck sl", ck=CK
                        ),
                    )

                def emit_qk(sec, hh):
                    nb = sec * NH + hh
                    wt = ph1wqk.tile([128, CK, 128], BF16, tag="wqk", name="wqk")
                    dma_engs[nb % 2].dma_start(
                        out=wt,
                        in_=wqk_d[:, nb * CK * 128 : (nb + 1) * CK * 128].rearrange(
                            "p (ck n) -> p ck n", ck=CK
                        ),
                    )
                    dest = q_sb if sec == 0 else k_sb
                    for st in range(NST):
                        ps = ps1.tile([128, 512], F32, tag="psqk", bufs=4, name="psqk")
                        for ck in range(CK):
                            nc.tensor.matmul(
                                ps,
                                lhsT=wt[:, ck, :],
                                rhs=xfull[:, ck, bass.ts(st, 512)],
                                start=(ck == 0),
                                stop=(ck == CK - 1),
                            )
                        # copy + per-partition bias add on ScalarE
                        nc.scalar.add(
                            dest[:, hh, bass.ts(st, 512)], ps, bias_col[:, nb : nb + 1]
                        )

                # v: n-tiles of 256, psum[s 128, n 256]
                for nt in range(NVT):
                    wt = wt0 if nt == 0 else load_wv(nt, [nc.gpsimd, nc.gpsimd])
                    for sc in range(NSC):
                        ps = ps1.tile([128, 512], F32, tag="psv", bufs=4)
                        psv = ps[:, :256]
                        for ck in range(CK):
                            nc.tensor.matmul(
                                psv,
                                lhsT=xfull[:, ck, bass.ts(sc, 128)],
                                rhs=wt[:, ck, :],
                                start=(ck == 0),
                                stop=(ck == CK - 1),
                            )
                        # copy + broadcast bias add on DVE
                        nc.vector.tensor_add(
                            v_sb[:, sc, bass.ts(nt, 256)],
                            psv,
                            bvb_sb[:, bass.ts(nt, 256)],
                        )
                # k then q, transposed: psum[n 128, s 512]
                for hh in range(NH):
                    emit_qk(1, hh)
                for hh in range(NH):
                    emit_qk(0, hh)

            # -------- Phase 2+3: block-causal attention + projection --------
            with (
                tc.tile_pool(name="att", bufs=4) as att,
                tc.tile_pool(name="ph3", bufs=2) as ph3,
                tc.tile_pool(name="ps2", bufs=1, space="PSUM") as ps2,
            ):
                wp = ph3.tile([128, NH, C], BF16, tag="wp", bufs=1)
                wp_src = wp_d.rearrange("p (h e) -> p h e", h=NH)
                nc.sync.dma_start(out=wp[:, : NH // 2, :], in_=wp_src[:, : NH // 2, :])
                nc.gpsimd.dma_start(out=wp[:, NH // 2 :, :], in_=wp_src[:, NH // 2 :, :])

                out_engs = [nc.sync, nc.gpsimd]

                def emit_proj(t_src, lo, hi, tag="po", bufs=1):
                    tiles = [
                        (sqc, et)
                        for sqc in range(4 * t_src, 4 * (t_src + 1))
                        for et in range(ET)
                    ]
                    for sqc, et in tiles[lo:hi]:
                        ps_o = ps2.tile([128, 512], F32, tag=tag, bufs=bufs)
                        for hp in range(NH):
                            nc.tensor.matmul(
                                ps_o,
                                lhsT=q_sb[:, hp, bass.ts(sqc, 128)],
                                rhs=wp[:, hp, bass.ts(et, 512)],
                                start=(hp == 0),
                                stop=(hp == NH - 1),
                            )
                        o_sb = ph3.tile([128, 512], F32, tag="o")
                        nc.vector.tensor_copy(o_sb, ps_o)
                        out_engs[(sqc * ET + et) % 2].dma_start(
                            out=out_d[bass.ts(sqc, 128), bass.ts(et, 512)], in_=o_sb
                        )

                for t in range(NST):
                    tsl = bass.ts(t, SQT)
                    nsk = 4 * t + 4  # block-causal sk chunks
                    pending = None   # previous head awaiting normalization

                    def flush_pending():
                        nonlocal pending
                        if pending is None:
                            return
                        yu_p, rs_p, h_p = pending
                        ps_bc = ps2.tile([128, 512], F32, tag="bc", bufs=1)
                        nc.tensor.matmul(
                            ps_bc, lhsT=ones_row_bf, rhs=rs_p, start=True, stop=True
                        )
                        bc_sb = att.tile([128, 512], F32, tag="bcs", bufs=2)
                        nc.vector.tensor_copy(bc_sb, ps_bc)
                        # yT (bf16) overwrites q_sb[:, h_p, tsl]
                        nc.vector.tensor_mul(q_sb[:, h_p, tsl], yu_p, bc_sb)
                        pending = None

                    for h in range(NH):
                        ps_yu = ps2.tile([128, 512], F32, tag="yu", bufs=2)
                        ps_rs = ps2.tile([1, 512], F32, tag="rs", bufs=1)
                        # row-sum partials on DVE; two interleaved chains for long
                        # blocks so the serial adds stay shorter than PE's work
                        acc = att.tile([128, 512], BF16, tag="acc", bufs=2, name="acc")
                        acc2 = None
                        if nsk > 8:
                            acc2 = att.tile([128, 512], BF16, tag="acc2", bufs=2, name="acc2")
                        sc_tiles = {}

                        def emit_scores(j, h=h):
                            off = 0 if j < 4 * t else (j - 4 * t) * 128
                            w = 512 - off
                            ps_sc = ps2.tile([128, 512], F32, tag="sc", bufs=3)
                            # scoresT[sk, sq] = k_h.T q_h (live sq columns only)
                            nc.tensor.matmul(
                                ps_sc[:, :w],
                                lhsT=k_sb[:, h, bass.ts(j, 128)],
                                rhs=q_sb[:, h, t * SQT + off : (t + 1) * SQT],
                                start=True,
                                stop=True,
                            )
                            sc_tiles[j] = (ps_sc, off, w)

                        emit_scores(0)
                        if nsk > 1:
                            emit_scores(1)
                        for j in range(nsk):
                            ps_sc, off, w = sc_tiles.pop(j)
                            e = att.tile([128, 512], BF16, tag="e", bufs=10)
                            nc.scalar.activation(
                                out=e[:, off:], in_=ps_sc[:, :w], func=ExpF, scale=scale
                            )
                            if j >= 4 * t:
                                # causal mask for the diagonal 128-block on DVE
                                nc.vector.tensor_mul(
                                    e[:, off : off + 128],
                                    e[:, off : off + 128],
                                    utri_sb,
                                )
                            if j + 2 < nsk:
                                emit_scores(j + 2)
                            if j == 0:
                                flush_pending()
                            # row sums (live region; first touch is full width)
                            tgt = acc if (acc2 is None or j % 2 == 0) else acc2
                            if j <= (0 if acc2 is None else 1):
                                nc.vector.tensor_copy(tgt, e)
                            else:
                                nc.vector.tensor_add(
                                    tgt[:, off:], tgt[:, off:], e[:, off:]
                                )
                            # yu[d, sq] += v[sk, d].T @ e[sk, sq] (live region)
                            nc.tensor.matmul(
                                ps_yu[:, off:],
                                lhsT=v_sb[:, j, bass.ts(h, 128)],
                                rhs=e[:, off:],
                                start=(j == 0),
                                stop=(j == nsk - 1),
                            )
                        # partition-reduce the accumulated exp sums on PE
                        nc.tensor.matmul(
                            ps_rs,
                            lhsT=ones_col_bf,
                            rhs=acc,
                            start=True,
                            stop=(acc2 is None),
                        )
                        if acc2 is not None:
                            nc.tensor.matmul(
                                ps_rs, lhsT=ones_col_bf, rhs=acc2, start=False, stop=True
                            )
                        rs_sb = att.tile([1, 512], BF16, tag="rsb", bufs=2)
                        with nc.allow_low_precision(reason="bf16 1/rowsum: 0.4% rel"):
                            nc.vector.reciprocal(rs_sb, ps_rs)
                        # interleave prev t-block's projection tiles: fills PE
                        # while this head's reciprocal completes on DVE
                        if t > 0:
                            emit_proj(t - 1, 2 * h, 2 * h + 2)
                        pending = (ps_yu, rs_sb, h)
                    flush_pending()
                    if t == NST - 1:
                        emit_proj(t, 0, 4 * ET, tag="yu", bufs=2)

    if compile:
        nc.compile()
    return nc


def _make_utri():
    """utri[p, f] = 1 if p <= f else 0 (keep sk<=sq within diag block)."""
    return np.triu(np.ones((128, 128), np.float32)).astype(NPBF16)


_NC_CACHE = None


def _get_nc():
    global _NC_CACHE
    if _NC_CACHE is None:
        _NC_CACHE = _build()
    return _NC_CACHE


def _make_in_maps(x, w_qkv, b_qkv, w_proj):
    utri = _make_utri()
    CK = C // 128
    in_maps = []
    for core in range(N_CORES):
        b = core // 2
        g = core % 2
        cs = slice(g * NQ, (g + 1) * NQ)
        xb = np.asarray(x[b], np.float32).astype(NPBF16)
        # xh[p, sc, ck, sl] = x[sc*128+sl, ck*128+p], flattened to [128, S*C/128]
        xT = np.ascontiguousarray(
            xb.reshape(S // 128, 128, C // 128, 128).transpose(3, 0, 2, 1)
        ).reshape(128, (S // 128) * (C // 128) * 128)
        wqkv_c = np.concatenate(
            [w_qkv[:, cs], w_qkv[:, C:][:, cs], w_qkv[:, 2 * C:][:, cs]], axis=1
        ).astype(NPBF16)
        # q/k weights: [p, nb, ck, n] with 128-col chunks
        wqk = np.ascontiguousarray(
            wqkv_c[:, : 2 * NQ]
            .reshape(CK, 128, 2 * NQ // 128, 128)
            .transpose(1, 2, 0, 3)
        ).reshape(128, -1)
        # v weights: [p, nt, ck, n] with 256-col chunks
        wv = np.ascontiguousarray(
            wqkv_c[:, 2 * NQ:]
            .reshape(CK, 128, NQ // 256, 256)
            .transpose(1, 2, 0, 3)
        ).reshape(128, -1)
        bqkv_c = np.concatenate(
            [b_qkv[cs], b_qkv[C:][cs], b_qkv[2 * C:][cs]]
        ).astype(NPBF16)
        bqkvcol = np.ascontiguousarray(
            bqkv_c[: 2 * NQ].reshape(2 * NQ // 128, 128).T
        )
        bvb = np.ascontiguousarray(
            np.broadcast_to(bqkv_c[2 * NQ:], (128, NQ))
        )
        # proj weights: [p, h, e]
        wp = np.ascontiguousarray(
            np.asarray(w_proj[cs, :], np.float32)
            .astype(NPBF16)
            .reshape(NH, 128, C)
            .transpose(1, 0, 2)
        ).reshape(128, -1)
        in_maps.append(
            {
                "xT": xT,
                "wqk": wqk,
                "wv": wv,
                "wp": wp,
                "bqkvcol": bqkvcol,
                "bvb": bvb,
                "utri": utri,
            }
        )
    return in_maps


def kernel(x, w_qkv, b_qkv, w_proj, b_proj):
    x = np.asarray(x, np.float32)
    w_qkv = np.asarray(w_qkv, np.float32)
    b_qkv = np.asarray(b_qkv, np.float32)
    w_proj = np.asarray(w_proj, np.float32)
    b_proj = np.asarray(b_proj, np.float32)

    nc = _get_nc()
    in_maps = _make_in_maps(x, w_qkv, b_qkv, w_proj)
    res = run_bass_kernel_spmd(nc, in_maps, core_ids=list(range(N_CORES)))

    out = np.empty((B, S, C), np.float32)
    for b in range(B):
        out[b] = res.results[2 * b]["out"] + res.results[2 * b + 1]["out"]
        out[b] += b_proj[None, :]
    return out
